# revision 12
# baseline (speedup 1.0000x reference)
"""Trainium2 Bass kernel for nn_CVXPolicy_Quadcopter.

Computes, for each of B=500000 samples:
    p = MLP(concat([t, z]));  c = [(p6+p7+p8)/m, p9, p10, p11]
    ustar = -c * exp(-0.5 * W(||c||^2))   (W = Lambert W, via Newton)

Design (pure data parallel over 8 cores, 63488 padded samples/core):
  - mm1 (PE): fp8e4m3 DoubleRow matmul, K=42 logical rows packed as
    [21, 2, *]: three 14-row blocks [x8; fp8(x8/16); fp8(2(x-x8))] against
    weights [W_hi; fp8(16(W-W_hi)); fp8(W_hi/2)] — error-compensated fp8
    (a-err ~3e-4 rms). 0.5 cyc/row -> ~13.7us/core.
  - tanh: split across engines at block (1024-sample) granularity.
    ACT blocks: native Tanh from PSUM. DVE blocks: one fused custom DVE op
    (deg-7 odd minimax poly, clamp folded into min(u, B2); input pre-scaled
    by s via a second fp8 weight set so the leading coeff is exactly -1).
  - mm2 (PE): per 128-sample chunk, lhsT = h[101, 128] (ones row folds
    b2), rhs = W2cn [101, 4] -> out [128, 4] PSUM: out free size 4 makes
    mm2 nearly free. c accumulates per group in PSUM [128, 4*chunks].
  - Lambert phase per group (pipelined into the next group's blocks):
    sq = Square(c) on ACT; x+1 via strided adds on Pool; W via
    exponent-bit init + 2 Newton iters (ACT exps, DVE recip/mul, Pool
    tensor ops); u = c * exp(-w/2) via 4 strided DVE muls -> fp16 out.
"""

import sys

import numpy as np

for _p in ("/opt/trn_rl_repo", "/root/.axon_site/_ro/trn_rl_repo"):
    if _p not in sys.path:
        sys.path.append(_p)

import ml_dtypes

_B = 500000
_NCORES = 8
_BLK = 1024              # samples per block (one tanh instruction)
_NBLK = 62               # blocks per core
_BLOC = _BLK * _NBLK     # 63488 per-core padded batch
_BPAD = _BLOC * _NCORES  # 507904
_GROUP_BLKS = (16, 16, 16, 14)
_NACT = 32               # of 62 blocks routed to ACT-tanh (rest: DVE poly)

_MASS = 0.5
# Lambert W: w0 = C*(int_bits(1+x) - B); 2 Newton iters
_LOG_B = 1064866805.0
_LOG_C = 6.197218803882235e-08
_NEWTON_ITERS = 2

# deg-7 odd minimax tanh on [-2,2] (Remez), leading coeff normalized to -1
# via input pre-scale s: tanh(a) ~ y*(c0 + u*(c1 + u*(c2 - u))),
# y = s*a, u = min(y*y, B2)
_PS = 0.4736539832117028
_PC0 = 2.083723576351597
_PC1 = -2.604200768651294
_PC2 = 2.472246731206437
_PB2 = 0.8975690857023427

_F8NP = ml_dtypes.float8_e4m3

_CACHE = {}


def _act_pattern():
    """Bresenham-interleave _NACT ACT blocks among _NBLK."""
    pat = []
    acc = 0
    for _ in range(_NBLK):
        acc += _NACT
        if acc >= _NBLK:
            acc -= _NBLK
            pat.append(True)
        else:
            pat.append(False)
    return pat


def _register_tanh7():
    from concourse import dve_ops
    from concourse.dve_spec import (C0, C1, C2, C3, Spec, Src0, lower, minn,
                                    sq, _has_src1, _spill_c3_to_src1)
    from concourse.dve_ops import DveOp
    from concourse.dve_uop import DveOpSpec

    for op in dve_ops.OPS:
        if op.name == "TANH7_ANT":
            return op

    u = sq(Src0)
    uc = minn(u, C2)
    body = Src0 * (C0 + uc * (C1 + uc * (C3 - uc)))

    def ref(in0, in1, s0, s1, imm2):
        x = np.asarray(in0, np.float32)
        ucv = np.minimum(x * x, np.float32(imm2))
        return x * (s0 + ucv * (s1 + ucv * (in1 - ucv)))

    spec = Spec(body=_spill_c3_to_src1(body), reference=ref)
    row = max(dve_ops._SUB_OPCODE_FOR_NAME.values()) + 1
    shas = {}
    for ver in ("v3", "v4"):
        tmp = DveOpSpec(name="TANH7_ANT", opcode=row,
                        uops=lower(spec, ver=ver), rd1_en=_has_src1(spec))
        shas[ver] = tmp.sha(ver)
    op = DveOp("TANH7_ANT", spec, subdim=False, uops_sha=shas)
    dve_ops.OPS.append(op)
    dve_ops.CUSTOM_DVE_SPECS[op.name] = op.spec
    dve_ops._SUB_OPCODE_FOR_NAME[op.name] = row
    return op


def _build_nc():
    import concourse.bacc as bacc
    import concourse.tile as tile
    from concourse import mybir

    f32 = mybir.dt.float32
    f16 = mybir.dt.float16
    f8 = mybir.dt.float8e4
    i32 = mybir.dt.int32
    AF = mybir.ActivationFunctionType
    ALU = mybir.AluOpType
    DR = mybir.MatmulPerfMode.DoubleRow

    tanh7 = _register_tanh7()

    nc = bacc.Bacc("TRN2", target_bir_lowering=False, debug=False,
                   num_devices=_NCORES)

    inp = nc.dram_tensor("inp", [21, 2 * _BLOC], f8, kind="ExternalInput")
    wdu = nc.dram_tensor("wdu", [21, 256], f8, kind="ExternalInput")
    wds = nc.dram_tensor("wds", [21, 256], f8, kind="ExternalInput")
    w2 = nc.dram_tensor("w2", [101, 4], f16, kind="ExternalInput")
    ones = nc.dram_tensor("ones", [1, _BLK], f16, kind="ExternalInput")
    out = nc.dram_tensor("out", [128, _NBLK * 32], f16, kind="ExternalOutput")

    pat = _act_pattern()
    # groups: block ranges and chunk offsets
    gb0 = []
    b0 = 0
    for n in _GROUP_BLKS:
        gb0.append(b0)
        b0 += n
    NG = len(_GROUP_BLKS)

    with tile.TileContext(nc) as tc:
        with (
            tc.tile_pool(name="consts", bufs=1) as consts,
            tc.tile_pool(name="inpp", bufs=2) as inp_pool,
            tc.tile_pool(name="hs", bufs=1) as h_pool,
            tc.tile_pool(name="sqp", bufs=2) as sq_pool,
            tc.tile_pool(name="lam", bufs=3) as lam_pool,
            tc.tile_pool(name="up", bufs=2) as u_pool,
            tc.tile_pool(name="mmp", bufs=1, space="PSUM") as mm_pool,
            tc.tile_pool(name="cps", bufs=1, space="PSUM") as c_pool,
        ):
            wdu_sb = consts.tile([21, 256], f8, tag="wdu")
            nc.sync.dma_start(wdu_sb, wdu[:])
            wds_sb = consts.tile([21, 256], f8, tag="wds")
            nc.sync.dma_start(wds_sb, wds[:])
            w2_sb = consts.tile([101, 4], f16, tag="w2")
            nc.sync.dma_start(w2_sb, w2[:])
            c2t = consts.tile([100, 1], f32, tag="c2t")
            nc.vector.memset(c2t, _PC2)
            half = consts.tile([128, 1], f32, tag="half")
            nc.vector.memset(half, 0.5)
            onep = consts.tile([128, 1], f32, tag="onep")
            nc.vector.memset(onep, 1.0)

            wdu_v = wdu_sb[:].rearrange("p (i m) -> p i m", i=2)[:, :, 0:100]
            wds_v = wds_sb[:].rearrange("p (i m) -> p i m", i=2)[:, :, 0:100]

            # h ring with persistent ones row (row 100) for the mm2 bias
            h_ring = [h_pool.tile([101, _BLK], f16, name=f"h{i}", tag=f"h{i}", bufs=1)
                      for i in range(3)]
            for ht in h_ring:
                nc.sync.dma_start(ht[100:101, :], ones[:])

            it_ring = [inp_pool.tile([21, 2 * 16384], f8, name=f"it{i}",
                                     tag=f"it{i}", bufs=1)
                       for i in range(2)]
            inp_v = inp[:].rearrange("p (i n) -> p i n", i=2)

            mm_ring = [mm_pool.tile([100, _BLK], f32, name=f"mm{i}", tag=f"mm{i}")
                       for i in range(2)]
            c_tiles = [c_pool.tile([128, 4 * 128 * n // 16], f32, name=f"c{g}", tag=f"c{g}")
                       for g, n in enumerate(_GROUP_BLKS)]

            def dma_group_in(g):
                n = _GROUP_BLKS[g] * _BLK
                s0 = gb0[g] * _BLK
                it = it_ring[g % 2]
                itv = it[:].rearrange("p (i n) -> p i n", i=2)
                nc.sync.dma_start(itv[:, :, 0:n], inp_v[:, :, s0:s0 + n])

            # deferred emission queue for the lambert phase
            pending = []

            def emit_some(k):
                for _ in range(k):
                    if pending:
                        pending.pop(0)()

            def lambert_group(g):
                """Emit the full per-group Lambert chain (closures)."""
                nch = _GROUP_BLKS[g] * _BLK // 128   # chunks
                cpt = c_tiles[g]
                sqt = sq_pool.tile([128, 4 * nch], f32, name=f"sq{g}", tag=f"sq{g}", bufs=1)
                yt = lam_pool.tile([128, nch], f32, name=f"y{g}", tag=f"y{g}", bufs=1)
                s2a = lam_pool.tile([128, nch], f32, name=f"s2a{g}", tag=f"s2a{g}", bufs=1)
                s2b = lam_pool.tile([128, nch], f32, name=f"s2b{g}", tag=f"s2b{g}", bufs=1)
                vt = [lam_pool.tile([128, nch], f32, name=f"v{g}_{i}", tag=f"v{g}_{i}", bufs=1)
                      for i in range(_NEWTON_ITERS + 1)]
                ft = [lam_pool.tile([128, nch], f32, name=f"f{g}_{i}", tag=f"f{g}_{i}", bufs=1)
                      for i in range(_NEWTON_ITERS)]
                rvt = [lam_pool.tile([128, nch], f32, name=f"rv{g}_{i}",
                                     tag=f"rv{g}_{i}", bufs=1)
                       for i in range(_NEWTON_ITERS)]
                pt = lam_pool.tile([128, nch], f32, name=f"p{g}", tag=f"p{g}", bufs=1)
                ttt = lam_pool.tile([128, nch], f32, name=f"tt{g}", tag=f"tt{g}", bufs=1)
                numt = lam_pool.tile([128, nch], f32, name=f"num{g}", tag=f"num{g}", bufs=1)
                net = lam_pool.tile([128, nch], f32, name=f"ne{g}", tag=f"ne{g}", bufs=1)
                ut = u_pool.tile([128, 4 * nch], f16, name=f"u{g}", tag=f"u{g}", bufs=1)

                sq4 = sqt[:].rearrange("p (n j) -> p n j", j=4)
                c4 = cpt[:].rearrange("p (n j) -> p n j", j=4)
                u4 = ut[:].rearrange("p (n j) -> p n j", j=4)
                s2a3 = s2a[:].rearrange("p (n j) -> p n j", j=1)
                s2b3 = s2b[:].rearrange("p (n j) -> p n j", j=1)

                pending.append(lambda: nc.scalar.activation(
                    sqt, cpt[:], AF.Square))
                pending.append(lambda: nc.vector.tensor_add(
                    s2a3, sq4[:, :, 0:1], sq4[:, :, 1:2]))
                pending.append(lambda: nc.vector.tensor_add(
                    s2b3, sq4[:, :, 2:3], sq4[:, :, 3:4]))
                # y = (1 + s2a) + s2b = ||c||^2 + 1
                pending.append(lambda: nc.vector.scalar_tensor_tensor(
                    yt, s2a, 1.0, s2b, op0=ALU.add, op1=ALU.add))
                # v0 = C*float(bits(y)) - (B*C - 1)
                pending.append(lambda: nc.scalar.activation(
                    vt[0], yt[:].bitcast(i32), AF.Copy,
                    scale=_LOG_C, bias=-( _LOG_B * _LOG_C - 1.0)))
                for i in range(_NEWTON_ITERS):
                    v, vn = vt[i], vt[i + 1]
                    f, rv = ft[i], rvt[i]
                    # f = exp(1 - v)
                    pending.append(lambda v=v, f=f: nc.scalar.activation(
                        f, v, AF.Exp, scale=-1.0, bias=onep[:]))
                    pending.append(lambda v=v, rv=rv: nc.vector.reciprocal(
                        rv, v))
                    # p = (1 - v)*v
                    pending.append(lambda v=v: nc.vector.scalar_tensor_tensor(
                        pt, v, 1.0, v, op0=ALU.subtract, op1=ALU.mult))
                    # tt = (y - 1)*f = x*e^(1-v)
                    pending.append(lambda f=f: nc.vector.scalar_tensor_tensor(
                        ttt, yt, -1.0, f, op0=ALU.add, op1=ALU.mult))
                    # num = (1 + p) + tt
                    pending.append(lambda: nc.vector.scalar_tensor_tensor(
                        numt, pt, 1.0, ttt, op0=ALU.add, op1=ALU.add))
                    pending.append(lambda vn=vn, rv=rv: nc.vector.tensor_mul(
                        vn, numt, rv))
                # ne = exp(-0.5*v + 0.5)
                pending.append(lambda: nc.scalar.activation(
                    net, vt[_NEWTON_ITERS], AF.Exp, scale=-0.5,
                    bias=half[:]))
                ne3 = net[:].rearrange("p (n j) -> p n j", j=1)
                for j in range(4):
                    pending.append(lambda j=j: nc.vector.tensor_mul(
                        u4[:, :, j:j + 1], c4[:, :, j:j + 1], ne3))
                pending.append(lambda: nc.sync.dma_start(
                    out[:, 32 * gb0[g]:32 * (gb0[g] + _GROUP_BLKS[g])], ut))

            dma_group_in(0)
            g_of_blk = []
            for g, n in enumerate(_GROUP_BLKS):
                g_of_blk += [g] * n

            for gb in range(_NBLK + 1):
                if gb < _NBLK:
                    g = g_of_blk[gb]
                    if gb == gb0[g] and g + 1 < NG:
                        dma_group_in(g + 1)
                    # mm1 for this block (one block ahead of mm2)
                    hp = mm_ring[gb % 2]
                    it = it_ring[g % 2]
                    itv = it[:].rearrange("p (i n) -> p i n", i=2)
                    base = (gb - gb0[g]) * _BLK
                    wsel = wdu_v if pat[gb] else wds_v
                    for t in range(4):
                        o = base + 256 * t
                        nc.tensor.matmul(
                            hp[:, 256 * t:256 * (t + 1)],
                            lhsT=wsel,
                            rhs=itv[:, :, o:o + 256],
                            start=True, stop=True, perf_mode=DR)
                if gb >= 1:
                    # mm2 for previous block
                    pb = gb - 1
                    pg = g_of_blk[pb]
                    ht = h_ring[pb % 3]
                    cpt = c_tiles[pg]
                    cbase = (pb - gb0[pg]) * 32
                    for ch in range(8):
                        nc.tensor.matmul(
                            cpt[:, cbase + 4 * ch:cbase + 4 * ch + 4],
                            lhsT=ht[0:101, 128 * ch:128 * ch + 128],
                            rhs=w2_sb[:],
                            start=True, stop=True)
                    if pb == gb0[pg] + _GROUP_BLKS[pg] - 1:
                        lambert_group(pg)
                if gb < _NBLK:
                    ht = h_ring[gb % 3]
                    if pat[gb]:
                        nc.scalar.activation(ht[0:100, :], hp[:], AF.Tanh)
                    else:
                        nc.vector._custom_dve(
                            tanh7, out=ht[0:100, :], in0=hp[:], in1=c2t[:],
                            s0=_PC0, s1=_PC1, imm2=_PB2)
                    emit_some(2)
            emit_some(len(pending))

    nc.compile()
    return nc


def _get_nc():
    if "nc" not in _CACHE:
        _CACHE["nc"] = _build_nc()
    return _CACHE["nc"]


def _host_prep(z, t, W1, b1, W2, b2):
    f32 = np.float32
    z = np.asarray(z, f32)
    t = np.asarray(t, f32)
    W1 = np.asarray(W1, f32)
    b1 = np.asarray(b1, f32)
    W2 = np.asarray(W2, f32)
    b2 = np.asarray(b2, f32)

    def q8(v):
        return np.asarray(v, _F8NP)

    xt = np.zeros((14, _BPAD), f32)
    xt[0, :_B] = t
    xt[1:13, :_B] = z.T
    xt[13, :] = 1.0
    x8 = q8(xt)
    x8f = x8.astype(f32)
    s8 = q8(x8f / 16.0)
    r8 = q8(2.0 * (xt - x8f))
    # [42, BPAD] logical rows -> [21, 2, BPAD]
    inp_dr = np.stack([np.concatenate([x8[0:14], s8[0:7]], axis=0),
                       np.concatenate([s8[7:14], r8[0:14]], axis=0)], axis=1)
    assert inp_dr.shape == (21, 2, _BPAD)

    W1a = np.concatenate([W1, b1[None, :]], axis=0)  # [14, 100]

    def wpack(Wm):
        W_hi = q8(Wm)
        W_hif = W_hi.astype(f32)
        W_lo = q8(16.0 * (Wm - W_hif))
        V = q8(W_hif / 2.0)
        Wfull = np.concatenate([W_hi, W_lo, V], axis=0)  # [42, 100]
        wp = np.zeros((21, 2, 128), _F8NP)
        wp[:, :, 0:100] = Wfull.reshape(2, 21, 100).transpose(1, 0, 2)
        return np.ascontiguousarray(wp.reshape(21, 256))

    wdu = wpack(W1a)
    wds = wpack(_PS * W1a)

    w2 = np.zeros((101, 4), np.float16)
    w2[0:100, 0] = (-(W2[:, 6] + W2[:, 7] + W2[:, 8]) / _MASS).astype(
        np.float16)
    w2[0:100, 1] = -W2[:, 9].astype(np.float16)
    w2[0:100, 2] = -W2[:, 10].astype(np.float16)
    w2[100, 0] = np.float16(-(b2[6] + b2[7] + b2[8]) / _MASS)
    w2[100, 1] = np.float16(-b2[9])
    w2[100, 2] = np.float16(-b2[10])
    w2[100, 3] = np.float16(-b2[11])
    w2[0:100, 3] = -W2[:, 11].astype(np.float16)

    ones16 = np.ones((1, _BLK), np.float16)
    return inp_dr, wdu, wds, w2, ones16


def kernel(z, t, W1, b1, W2, b2):
    from concourse.bass_utils import run_bass_kernel_spmd

    inp_dr, wdu, wds, w2, ones16 = _host_prep(z, t, W1, b1, W2, b2)
    nc = _get_nc()

    in_maps = []
    for c in range(_NCORES):
        sl = inp_dr[:, :, _BLOC * c:_BLOC * (c + 1)]
        in_maps.append({
            "inp": np.ascontiguousarray(sl).reshape(21, 2 * _BLOC),
            "wdu": wdu,
            "wds": wds,
            "w2": w2,
            "ones": ones16,
        })

    res = run_bass_kernel_spmd(nc, in_maps, core_ids=list(range(_NCORES)))

    gb0 = []
    b0 = 0
    for n in _GROUP_BLKS:
        gb0.append(b0)
        b0 += n

    u = np.empty((_BPAD, 4), np.float32)
    for c in range(_NCORES):
        o = res.results[c]["out"]  # [128, 1984] f16
        for g, n in enumerate(_GROUP_BLKS):
            nch = n * _BLK // 128
            blk = o[:, 32 * gb0[g]:32 * gb0[g] + 4 * nch].astype(np.float32)
            part = blk.reshape(128, nch, 4).transpose(1, 0, 2).reshape(-1, 4)
            s0 = c * _BLOC + gb0[g] * _BLK
            u[s0:s0 + n * _BLK] = part
    return np.ascontiguousarray(u[:_B])
